# revision 13
# baseline (speedup 1.0000x reference)
"""Trainium2 Bass kernel for causal multi-head attention (B=4, T=2048, C=1024, H=16).

Sharding: tensor-parallel over heads x batch. 8 cores = 4 batches x 2 head-halves.
Each core computes, for its batch b and its 8 heads:
  qkv projection -> causal attention -> output projection partial (rows of w_proj)
Host gathers by summing the two half-partials per batch (the "all-reduce").

Schedule (v2): single fused instruction stream keeps the PE dense so the HAM
clock gate stays warm (2.4 GHz) and the scalar engine (exp) is fed from ~30us:
  A: Q,K projections for t<1024, then V for t<1024.
  B: per head, attention for q<1024 (scores pipelined one kt ahead of exp->PV),
     interleaved with Q,K,V projection groups for t>=1024 as PE filler.
  C: per head, attention for q>=1024, interleaved with out-proj tiles t<1024.
  D: out-proj tiles t>=1024.
Initial DMAs are split per-128-row slice across both HWDGE queues (sync + act)
so the first matmul starts ~2us after the preamble instead of ~23us.

Per-core layouts / precision:
  x slices  [128, 512] f32r    contraction dim c on partitions.
  Q^T, K^T as [j=512, T] bf16  (4 partition-tiles of 2 heads each). K is
  pre-scaled by log2(e)/8 so scores S' = S*log2(e)/8 and exp via 2^{S'}
  (scalar activation Exp with scale=ln2). Scores are computed transposed:
  S'^T[k, q] = sum_d K'^T[d,k] Q^T[d,q], so P^T feeds the PV matmul directly.
  V as [t, h, 65] bf16 with a ones column per head: row 64 of the PV output is
  the softmax denominator; the division happens after an unnormalized copy of
  the PV psum into y (frees the psum bank early), via a DRAM-bounce broadcast.
  Causal masks (diagonal 128x128 blocks) multiply on the gpsimd engine.
  QKV projections and the output projection contract in fp32r; attention
  matmuls run in bf16; all PSUM accumulation is fp32.
"""

import sys

for _p in ("/opt/trn_rl_repo",):
    if _p not in sys.path:
        sys.path.insert(0, _p)

import math

import numpy as np

import concourse.bass as bass
import concourse.mybir as mybir
import concourse.tile as tile
from concourse import bacc
from concourse.bass import ts
from concourse.bass_utils import run_bass_kernel_spmd

B, T, C, H, D = 4, 2048, 1024, 16, 64
NCORES = 8
JC = 512  # channels per core (8 heads x 64)
HL = 8  # heads per core
CT = C // 128  # 8 contraction tiles
TT = T // 128  # 16 t(=k) tiles
TCH = 512  # projection t-chunk
NCH = T // TCH  # 4 chunks
F32 = mybir.dt.float32
F32R = mybir.dt.float32r
BF16 = mybir.dt.bfloat16
EXP = mybir.ActivationFunctionType.Exp
ADD = mybir.AluOpType.add
MULT = mybir.AluOpType.mult
KSCALE = math.log2(math.e) / 8.0  # folded into K so probs = 2^{S'}
LN2 = math.log(2.0)


def _r(ap):
    return ap.bitcast(F32R)


class _Ctx:
    pass


def _proj_group(g, which, c, jt):
    """One projection psum group: 8 accumulating matmuls + drain.

    which in {q, k, v}; c = t-chunk; jt = output 128-slice (for q/k) or
    t-sub-block (for v).
    """
    nc = g.nc
    if which == "v":
        # out tile [128 t, JC j] for t-sub-block jt of chunk c
        tt = c * (TCH // 128) + jt
        ps = g.pp.tile([128, JC], F32, tag="pp", name=f"pv_ps{tt}")
        for ct in range(CT):
            nc.tensor.matmul(
                ps,
                lhsT=g.xt[c][:, ct, ts(jt, 128)],
                rhs=g.wv[:, ct, :],
                start=(ct == 0),
                stop=(ct == CT - 1),
            )
        nc.vector.tensor_tensor(
            out=g.v_sb[tt][:, :, 0:64],
            in0=ps.rearrange("p (h d) -> p h d", h=HL),
            in1=g.bv_sb.rearrange("p (h d) -> p h d", h=HL),
            op=ADD,
        )
    else:
        w, dst, bcol = (g.wq, g.q_sb, g.bq_sb) if which == "q" else (g.wk, g.k_sb, g.bk_sb)
        ps = g.pp.tile([128, TCH], F32, tag="pp", name=f"p{which}{c}_{jt}")
        for ct in range(CT):
            nc.tensor.matmul(
                ps,
                lhsT=w[:, ct, ts(jt, 128)],
                rhs=g.xt[c][:, ct, :],
                start=(ct == 0),
                stop=(ct == CT - 1),
            )
        if which == "q":
            for piece in range(2):
                rows = slice(64 * piece, 64 * piece + 64)
                nc.vector.tensor_scalar_add(
                    out=dst[2 * jt + piece][rows, ts(c, TCH)],
                    in0=ps[rows, :],
                    scalar1=bcol[rows, jt : jt + 1],
                )
        else:
            nc.vector.tensor_scalar(
                out=dst[jt][:, ts(c, TCH)],
                in0=ps,
                scalar1=bcol[:, jt : jt + 1],
                scalar2=KSCALE,
                op0=ADD,
                op1=MULT,
            )


def _oproj_tile(g, tt):
    """Output projection for t-block tt: y[:, tt] @ wp + bp -> out DMA."""
    nc = g.nc
    ot = g.o_pool.tile([128, C], F32, tag="o", name=f"ot{tt}")
    for ch in range(2):
        ps = g.pp.tile([128, 512], F32, tag="pp", name=f"op{tt}_{ch}")
        for jt in range(4):
            nc.tensor.matmul(
                ps,
                lhsT=g.y_sb[jt][:, ts(tt, 128)],
                rhs=g.wp_sb[:, jt, ts(ch, 512)],
                start=(jt == 0),
                stop=(jt == 3),
            )
        nc.vector.tensor_tensor(
            out=ot[:, ts(ch, 512)], in0=ps, in1=g.bp_sb[:, ts(ch, 512)], op=ADD
        )
    nc.sync.dma_start(out=g.out[ts(tt, 128), :], in_=ot)


def _attn_half(g, h, half, fillers):
    """Causal attention for head h, query range [1024*half, 1024*(half+1)).

    Scores for kt+1 are emitted before PV for kt so the PE never waits on the
    scalar engine's exp. `fillers` is a list of zero-arg emitters (projection /
    out-proj groups) drained at fixed points to fill PE time while exp runs.
    """
    nc = g.nc
    jt, hrow = h // 2, 64 * (h % 2)
    qlo, qhi = 1024 * half, 1024 * (half + 1)
    nkt = qhi // 128
    fill_at = {2, 5, 9, 13}

    sc_t = {}
    pt_t = {}

    def emit_scores(kt):
        qstart = max(qlo, 128 * kt)
        w = qhi - qstart
        scps = g.sc.tile([128, 1024], F32, tag="sc", name=f"sc{h}_{half}_{kt}")
        sc_t[kt] = (scps, qstart, w)
        for q5 in range(0, w, 512):
            w5 = min(512, w - q5)
            nc.tensor.matmul(
                scps[:, q5 : q5 + w5],
                lhsT=g.k_sb[jt][:, ts(kt, 128)],
                rhs=g.q_sb[h][:, qstart + q5 : qstart + q5 + w5],
                start=True,
                stop=True,
            )

    def emit_exp(kt):
        scps, qstart, w = sc_t[kt]
        pt = g.pt.tile([128, 1024], BF16, tag="pt", name=f"pt{h}_{half}_{kt}")
        pt_t[kt] = pt
        nc.scalar.activation(out=pt[:, 0:w], in_=scps[:, 0:w], func=EXP, scale=LN2)
        if qstart == 128 * kt:
            nc.gpsimd.tensor_tensor(
                out=pt[:, 0:128], in0=pt[:, 0:128], in1=g.tri_sb, op=MULT
            )

    def emit_pv(kt):
        _, qstart, _ = sc_t[kt]
        pt = pt_t.pop(kt)
        for qb in range(2):
            blo = qlo + 512 * qb
            bhi = blo + 512
            lo = max(qstart, blo)
            if lo >= bhi:
                continue
            last_kt = bhi // 128 - 1
            nc.tensor.matmul(
                g.pv[qb][:, lo - blo : 512],
                lhsT=g.v_sb[kt][:, h, :],
                rhs=pt[:, lo - qstart : bhi - qstart],
                start=(kt == 0),
                stop=(kt == last_kt),
            )
            if kt == last_kt:
                # unnormalized drain frees the psum bank for the next head
                nc.vector.tensor_copy(
                    out=g.y_sb[jt][hrow : hrow + 64, blo:bhi], in_=g.pv[qb][0:64, :]
                )
                nc.vector.tensor_copy(
                    out=g.den[0:1, ts(qb, 512)], in_=g.pv[qb][64:65, :]
                )

    g.pv = [
        g.pv_pool.tile([128, 512], F32, tag=f"pv{qb}", name=f"pv{h}_{half}_{qb}")
        for qb in range(2)
    ]  # bufs=1 per tag: 2 PSUM banks total; early unnormalized drain frees them
    g.den = g.rd.tile([1, 1024], F32, tag="den", name=f"dn{h}{half}")
    g.rden = g.rd.tile([1, 1024], F32, tag="rden", name=f"rr{h}{half}")

    fq = list(fillers)
    prev = None
    for kt in range(nkt):
        emit_scores(kt)
        emit_exp(kt)
        if prev is not None:
            emit_pv(prev)
        if kt in fill_at and fq:
            fq.pop(0)()
        prev = kt
    emit_pv(prev)
    for f in fq:
        f()

    # normalize: rank-1 matmul broadcasts 1/den across partitions (no DRAM)
    nc.vector.reciprocal_approx_fast(out=g.rden, in_=g.den)
    rdenb = g.rd.tile([1, 1024], BF16, tag="rdenb", name=f"rc{h}{half}")
    nc.vector.tensor_copy(out=rdenb, in_=g.rden)
    brps = g.sc.tile([128, 1024], F32, tag="sc", name=f"br{h}_{half}")
    for qb in range(2):
        nc.tensor.matmul(
            brps[:, ts(qb, 512)],
            lhsT=g.ones1,
            rhs=rdenb[0:1, ts(qb, 512)],
            start=True,
            stop=True,
        )
    nc.vector.tensor_tensor(
        out=g.y_sb[jt][hrow : hrow + 64, qlo:qhi],
        in0=g.y_sb[jt][hrow : hrow + 64, qlo:qhi],
        in1=brps[hrow : hrow + 64, :],
        op=MULT,
    )


def _trace(nc, tc, io):
    xT, wq, wk, wv, wp, bq, bk, bv, bp, tri, out = io
    g = _Ctx()
    g.nc = nc
    g.out = out

    with (
        tc.tile_pool(name="consts", bufs=1) as consts,
        tc.tile_pool(name="qk", bufs=1) as qk_pool,
        tc.tile_pool(name="vp", bufs=1) as v_pool,
        tc.tile_pool(name="yp", bufs=1) as y_pool,
        tc.tile_pool(name="pt", bufs=3) as pt_pool,
        tc.tile_pool(name="rd", bufs=2) as rd_pool,
        tc.tile_pool(name="osb", bufs=3) as o_pool,
        tc.tile_pool(name="wpp", bufs=1) as wp_pool,
        tc.tile_pool(name="pp", bufs=2, space="PSUM") as pp_pool,
        tc.tile_pool(name="sc", bufs=2, space="PSUM") as sc_pool,
        tc.tile_pool(name="pvp", bufs=1, space="PSUM") as pv_pool,
    ):
        g.pt, g.rd, g.o_pool = pt_pool, rd_pool, o_pool
        g.pp, g.sc, g.pv_pool = pp_pool, sc_pool, pv_pool

        # ---- constants (scalar queue; tiny) -----------------------------
        g.tri_sb = consts.tile([128, 128], BF16, tag="tri")
        nc.scalar.dma_start(out=g.tri_sb, in_=tri)
        g.bq_sb = consts.tile([128, 4], F32, tag="bq")
        nc.scalar.dma_start(out=g.bq_sb, in_=bq.rearrange("(jt p) -> p jt", p=128))
        g.bk_sb = consts.tile([128, 4], F32, tag="bk")
        nc.scalar.dma_start(out=g.bk_sb, in_=bk.rearrange("(jt p) -> p jt", p=128))
        g.bv_sb = consts.tile([128, JC], F32, tag="bv")
        nc.scalar.dma_start(out=g.bv_sb, in_=bv.unsqueeze(0).to_broadcast([128, JC]))
        g.bp_sb = consts.tile([128, C], F32, tag="bp")
        nc.scalar.dma_start(out=g.bp_sb, in_=bp.unsqueeze(0).to_broadcast([128, C]))
        g.ones1 = consts.tile([1, 128], BF16, tag="ones1")
        nc.vector.memset(g.ones1, 1.0)

        g.q_sb = [qk_pool.tile([128, T], BF16, tag=f"q{h}", name=f"q{h}") for h in range(HL)]
        g.k_sb = [qk_pool.tile([128, T], BF16, tag=f"k{jt}", name=f"k{jt}") for jt in range(4)]
        g.v_sb = [v_pool.tile([128, HL, 128], BF16, tag=f"v{tt}", name=f"v{tt}") for tt in range(TT)]
        g.y_sb = [y_pool.tile([128, T], BF16, tag=f"y{jt}", name=f"y{jt}") for jt in range(4)]
        # zero-pad the unused half of each per-head Q tile (full-128 contraction
        # keeps the PE array activity high so the HAM clock gate stays warm)
        for h in range(HL):
            pad = (64, 128) if h % 2 == 0 else (0, 64)
            nc.vector.memset(g.q_sb[h][pad[0] : pad[1], :], 0.0)
        for tt in range(TT):
            nc.vector.memset(g.v_sb[tt], 0.0)
            nc.vector.memset(g.v_sb[tt][:, :, 64:65], 1.0)

        with (
            tc.tile_pool(name="wat", bufs=1) as w_pool,
            tc.tile_pool(name="xt", bufs=2) as xt_pool,
        ):
            # ---- initial DMAs: interleave across both HWDGE queues ------
            # scalar queue: wq, wk slices, wp (done before the first exp).
            # sync queue: xt chunks, wv, then bounces + out stores.
            xT_r = xT.rearrange("(ct p) t -> p ct t", p=128)
            g.wq = w_pool.tile([128, CT, JC], BF16, tag="wq")
            g.wk = w_pool.tile([128, CT, JC], BF16, tag="wk")
            g.wv = w_pool.tile([128, CT, JC], BF16, tag="wv")
            g.xt = [
                xt_pool.tile([128, CT, TCH], BF16, tag="xt", name=f"x{c}")
                for c in range(NCH)
            ]
            nc.scalar.dma_start(out=g.wq, in_=wq.rearrange("(ct p) j -> p ct j", p=128))
            nc.sync.dma_start(out=g.xt[0], in_=xT_r[:, :, ts(0, TCH)])
            nc.scalar.dma_start(out=g.wk, in_=wk.rearrange("(ct p) j -> p ct j", p=128))
            nc.sync.dma_start(out=g.xt[1], in_=xT_r[:, :, ts(1, TCH)])
            g.wp_sb = wp_pool.tile([128, 4, C], BF16, tag="wp")
            nc.scalar.dma_start(
                out=g.wp_sb, in_=wp.rearrange("(jt p) c -> p jt c", p=128)
            )
            nc.sync.dma_start(out=g.wv, in_=wv.rearrange("(ct p) j -> p ct j", p=128))
            for c in (2, 3):
                nc.sync.dma_start(out=g.xt[c], in_=xT_r[:, :, ts(c, TCH)])

            # ---- phase A: QK then V for t < 1024 ------------------------
            for c in (0, 1):
                for which in ("q", "k"):
                    for jt in range(4):
                        _proj_group(g, which, c, jt)
            for c in (0, 1):
                for sub in range(TCH // 128):
                    _proj_group(g, "v", c, sub)

            # ---- phase B: attention half 0 + projection fillers ---------
            fillers = []
            for c in (2, 3):
                for which in ("q", "k"):
                    for jt in range(4):
                        fillers.append(
                            (lambda wh=which, cc=c, j=jt: _proj_group(g, wh, cc, j))
                        )
            for c in (2, 3):
                for sub in range(TCH // 128):
                    fillers.append((lambda cc=c, s=sub: _proj_group(g, "v", cc, s)))
            per = (len(fillers) + HL - 1) // HL
            for h in range(HL):
                _attn_half(g, h, 0, fillers[h * per : (h + 1) * per])

        # ---- phase C: attention half 1 + out-proj t<1024 fillers --------
        for h in range(HL):
            _attn_half(g, h, 1, [lambda t0=h: _oproj_tile(g, t0)])

        # ---- phase D: out-proj t >= 1024 --------------------------------
        for tt in range(8, 16):
            _oproj_tile(g, tt)


_CACHE = {}


def build_nc():
    if "nc" in _CACHE:
        return _CACHE["nc"]
    nc = bacc.Bacc(
        "TRN2",
        target_bir_lowering=False,
        debug=False,
        enable_asserts=False,
        num_devices=NCORES,
    )
    io = (
        nc.dram_tensor("xT", [C, T], BF16, kind="ExternalInput").ap(),
        nc.dram_tensor("wq", [C, JC], BF16, kind="ExternalInput").ap(),
        nc.dram_tensor("wk", [C, JC], BF16, kind="ExternalInput").ap(),
        nc.dram_tensor("wv", [C, JC], BF16, kind="ExternalInput").ap(),
        nc.dram_tensor("wp", [JC, C], BF16, kind="ExternalInput").ap(),
        nc.dram_tensor("bq", [JC], F32, kind="ExternalInput").ap(),
        nc.dram_tensor("bk", [JC], F32, kind="ExternalInput").ap(),
        nc.dram_tensor("bv", [JC], F32, kind="ExternalInput").ap(),
        nc.dram_tensor("bp", [C], F32, kind="ExternalInput").ap(),
        nc.dram_tensor("tri", [128, 128], BF16, kind="ExternalInput").ap(),
        nc.dram_tensor("out", [T, C], F32, kind="ExternalOutput").ap(),
    )
    with tile.TileContext(nc) as tc:
        _trace(nc, tc, io)
    nc.compile()
    _CACHE["nc"] = nc
    return nc


def make_in_maps(x, w_attn, b_attn, w_proj, b_proj):
    import ml_dtypes
    tri = np.triu(np.ones((128, 128), dtype=ml_dtypes.bfloat16))
    zeros_c = np.zeros(C, dtype=np.float32)
    in_maps = []
    for core in range(NCORES):
        b, hh = core // 2, core % 2
        j0 = JC * hh
        in_maps.append(
            {
                "xT": np.ascontiguousarray(x[b].T).astype(ml_dtypes.bfloat16),
                "wq": np.ascontiguousarray(w_attn[:, j0 : j0 + JC]).astype(ml_dtypes.bfloat16),
                "wk": np.ascontiguousarray(w_attn[:, C + j0 : C + j0 + JC]).astype(ml_dtypes.bfloat16),
                "wv": np.ascontiguousarray(w_attn[:, 2 * C + j0 : 2 * C + j0 + JC]).astype(ml_dtypes.bfloat16),
                "wp": np.ascontiguousarray(w_proj[j0 : j0 + JC, :]).astype(ml_dtypes.bfloat16),
                "bq": np.ascontiguousarray(b_attn[j0 : j0 + JC]),
                "bk": np.ascontiguousarray(b_attn[C + j0 : C + j0 + JC]),
                "bv": np.ascontiguousarray(b_attn[2 * C + j0 : 2 * C + j0 + JC]),
                "bp": (b_proj.astype(np.float32) if hh == 0 else zeros_c),
                "tri": tri,
            }
        )
    return in_maps


def gather(parts):
    out = np.empty((B, T, C), dtype=np.float32)
    for b in range(B):
        out[b] = parts[2 * b]["out"] + parts[2 * b + 1]["out"]
    return out


def kernel(x, w_attn, b_attn, w_proj, b_proj):
    x = np.asarray(x, dtype=np.float32)
    w_attn = np.asarray(w_attn, dtype=np.float32)
    b_attn = np.asarray(b_attn, dtype=np.float32)
    w_proj = np.asarray(w_proj, dtype=np.float32)
    b_proj = np.asarray(b_proj, dtype=np.float32)
    nc = build_nc()
    in_maps = make_in_maps(x, w_attn, b_attn, w_proj, b_proj)
    res = run_bass_kernel_spmd(nc, in_maps, core_ids=list(range(NCORES)))
    return gather(res.results)


if __name__ == "__main__":
    rng = np.random.default_rng(0)
    x = rng.standard_normal((B, T, C), dtype=np.float32)
    w_attn = rng.standard_normal((C, 3 * C), dtype=np.float32) / np.sqrt(C)
    b_attn = np.zeros(3 * C, np.float32)
    w_proj = rng.standard_normal((C, C), dtype=np.float32) / np.sqrt(C)
    b_proj = np.zeros(C, np.float32)
    out = kernel(x, w_attn, b_attn, w_proj, b_proj)
    print(out.shape, out.dtype, np.abs(out).mean())


# revision 14
# speedup vs baseline: 1.1742x; 1.1742x over previous
"""Trainium2 Bass kernel for causal multi-head attention (B=4, T=2048, C=1024, H=16).

Sharding: tensor-parallel over heads x batch. 8 cores = 4 batches x 2 head-halves.
Each core computes, for its batch b and its 8 heads:
  qkv projection -> causal attention -> output projection partial (rows of w_proj)
Host gathers by summing the two half-partials per batch (the "all-reduce").

Schedule (v2): single fused instruction stream keeps the PE dense so the HAM
clock gate stays warm (2.4 GHz) and the scalar engine (exp) is fed from ~30us:
  A: Q,K projections for t<1024, then V for t<1024.
  B: per head, attention for q<1024 (scores pipelined one kt ahead of exp->PV),
     interleaved with Q,K,V projection groups for t>=1024 as PE filler.
  C: per head, attention for q>=1024, interleaved with out-proj tiles t<1024.
  D: out-proj tiles t>=1024.
Initial DMAs are split per-128-row slice across both HWDGE queues (sync + act)
so the first matmul starts ~2us after the preamble instead of ~23us.

Per-core layouts / precision:
  x slices  [128, 512] f32r    contraction dim c on partitions.
  Q^T, K^T as [j=512, T] bf16  (4 partition-tiles of 2 heads each). K is
  pre-scaled by log2(e)/8 so scores S' = S*log2(e)/8 and exp via 2^{S'}
  (scalar activation Exp with scale=ln2). Scores are computed transposed:
  S'^T[k, q] = sum_d K'^T[d,k] Q^T[d,q], so P^T feeds the PV matmul directly.
  V as [t, h, 65] bf16 with a ones column per head: row 64 of the PV output is
  the softmax denominator; the division happens after an unnormalized copy of
  the PV psum into y (frees the psum bank early), via a DRAM-bounce broadcast.
  Causal masks (diagonal 128x128 blocks) multiply on the gpsimd engine.
  QKV projections and the output projection contract in fp32r; attention
  matmuls run in bf16; all PSUM accumulation is fp32.
"""

import sys

for _p in ("/opt/trn_rl_repo",):
    if _p not in sys.path:
        sys.path.insert(0, _p)

import math

import numpy as np

import concourse.bass as bass
import concourse.mybir as mybir
import concourse.tile as tile
from concourse import bacc
from concourse.bass import ts
from concourse.bass_utils import run_bass_kernel_spmd

B, T, C, H, D = 4, 2048, 1024, 16, 64
NCORES = 8
JC = 512  # channels per core (8 heads x 64)
HL = 8  # heads per core
CT = C // 128  # 8 contraction tiles
TT = T // 128  # 16 t(=k) tiles
TCH = 512  # projection t-chunk
NCH = T // TCH  # 4 chunks
F32 = mybir.dt.float32
F32R = mybir.dt.float32r
BF16 = mybir.dt.bfloat16
EXP = mybir.ActivationFunctionType.Exp
ADD = mybir.AluOpType.add
MULT = mybir.AluOpType.mult
KSCALE = math.log2(math.e) / 8.0  # folded into K so probs = 2^{S'}
LN2 = math.log(2.0)


def _r(ap):
    return ap.bitcast(F32R)


class _Ctx:
    pass


def _proj_group(g, which, c, jt):
    """One projection psum group: 8 accumulating matmuls + drain.

    which in {q, k, v}; c = t-chunk; jt = output 128-slice (for q/k) or
    t-sub-block (for v).
    """
    nc = g.nc
    if which == "v":
        # out tile [128 t, JC j] for t-sub-block jt of chunk c
        tt = c * (TCH // 128) + jt
        ps = g.pp.tile([128, JC], F32, tag="pp", name=f"pv_ps{tt}")
        for ct in range(CT):
            nc.tensor.matmul(
                ps,
                lhsT=g.xt[c][:, ct, ts(jt, 128)],
                rhs=g.wv[:, ct, :],
                start=(ct == 0),
                stop=(ct == CT - 1),
            )
        nc.vector.tensor_tensor(
            out=g.v_sb[tt][:, :, 0:64],
            in0=ps.rearrange("p (h d) -> p h d", h=HL),
            in1=g.bv_sb.rearrange("p (h d) -> p h d", h=HL),
            op=ADD,
        )
    else:
        w, dst, bcol = (g.wq, g.q_sb, g.bq_sb) if which == "q" else (g.wk, g.k_sb, g.bk_sb)
        ps = g.pp.tile([128, TCH], F32, tag="pp", name=f"p{which}{c}_{jt}")
        for ct in range(CT):
            nc.tensor.matmul(
                ps,
                lhsT=w[:, ct, ts(jt, 128)],
                rhs=g.xt[c][:, ct, :],
                start=(ct == 0),
                stop=(ct == CT - 1),
            )
        if which == "q":
            for piece in range(2):
                rows = slice(64 * piece, 64 * piece + 64)
                nc.vector.tensor_scalar_add(
                    out=dst[2 * jt + piece][rows, ts(c, TCH)],
                    in0=ps[rows, :],
                    scalar1=bcol[rows, jt : jt + 1],
                )
        else:
            nc.vector.tensor_scalar(
                out=dst[jt][:, ts(c, TCH)],
                in0=ps,
                scalar1=bcol[:, jt : jt + 1],
                scalar2=KSCALE,
                op0=ADD,
                op1=MULT,
            )


def _oproj_tile(g, tt):
    """Output projection for t-block tt: y[:, tt] @ wp + bp -> out DMA."""
    nc = g.nc
    ot = g.o_pool.tile([128, C], F32, tag="o", name=f"ot{tt}")
    for ch in range(2):
        ps = g.pp.tile([128, 512], F32, tag="pp", name=f"op{tt}_{ch}")
        for jt in range(4):
            nc.tensor.matmul(
                ps,
                lhsT=g.y_sb[jt][:, ts(tt, 128)],
                rhs=g.wp_sb[:, jt, ts(ch, 512)],
                start=(jt == 0),
                stop=(jt == 3),
            )
        nc.vector.tensor_tensor(
            out=ot[:, ts(ch, 512)], in0=ps, in1=g.bp_sb[:, ts(ch, 512)], op=ADD
        )
    nc.sync.dma_start(out=g.out[ts(tt, 128), :], in_=ot)


def _attn_half(g, h, half, fillers):
    """Causal attention for head h, query range [1024*half, 1024*(half+1)).

    Scores for kt+1 are emitted before PV for kt so the PE never waits on the
    scalar engine's exp. `fillers` is a list of zero-arg emitters (projection /
    out-proj groups) drained at fixed points to fill PE time while exp runs.
    """
    nc = g.nc
    jt, hrow = h // 2, 64 * (h % 2)
    qlo, qhi = 1024 * half, 1024 * (half + 1)
    nkt = qhi // 128
    fill_at = {2, 5, 9, 13}

    sc_t = {}
    pt_t = {}

    def emit_scores(kt):
        qstart = max(qlo, 128 * kt)
        w = qhi - qstart
        scps = g.sc.tile([128, 1024], F32, tag="sc", name=f"sc{h}_{half}_{kt}")
        sc_t[kt] = (scps, qstart, w)
        for q5 in range(0, w, 512):
            w5 = min(512, w - q5)
            nc.tensor.matmul(
                scps[:, q5 : q5 + w5],
                lhsT=g.k_sb[jt][:, ts(kt, 128)],
                rhs=g.q_sb[h][:, qstart + q5 : qstart + q5 + w5],
                start=True,
                stop=True,
            )

    def emit_exp(kt):
        scps, qstart, w = sc_t[kt]
        pt = g.pt.tile([128, 1024], BF16, tag="pt", name=f"pt{h}_{half}_{kt}")
        pt_t[kt] = pt
        nc.scalar.activation(out=pt[:, 0:w], in_=scps[:, 0:w], func=EXP, scale=LN2)
        if qstart == 128 * kt:
            nc.gpsimd.tensor_tensor(
                out=pt[:, 0:128], in0=pt[:, 0:128], in1=g.tri_sb, op=MULT
            )

    def emit_pv(kt):
        _, qstart, _ = sc_t[kt]
        pt = pt_t.pop(kt)
        for qb in range(2):
            blo = qlo + 512 * qb
            bhi = blo + 512
            lo = max(qstart, blo)
            if lo >= bhi:
                continue
            last_kt = bhi // 128 - 1
            nc.tensor.matmul(
                g.pv[qb][:, lo - blo : 512],
                lhsT=g.v_sb[kt][:, h, :],
                rhs=pt[:, lo - qstart : bhi - qstart],
                start=(kt == 0),
                stop=(kt == last_kt),
            )
            if kt == last_kt:
                # unnormalized drain frees the psum bank for the next head
                nc.vector.tensor_copy(
                    out=g.y_sb[jt][hrow : hrow + 64, blo:bhi], in_=g.pv[qb][0:64, :]
                )
                nc.vector.tensor_copy(
                    out=g.den[0:1, ts(qb, 512)], in_=g.pv[qb][64:65, :]
                )

    g.pv = [
        g.pv_pool.tile([128, 512], F32, tag=f"pv{qb}", name=f"pv{h}_{half}_{qb}")
        for qb in range(2)
    ]  # bufs=1 per tag: 2 PSUM banks total; early unnormalized drain frees them
    g.den = g.rd.tile([1, 1024], F32, tag="den", name=f"dn{h}{half}")
    g.rden = g.rd.tile([1, 1024], F32, tag="rden", name=f"rr{h}{half}")

    fq = list(fillers)
    prev = None
    for kt in range(nkt):
        emit_scores(kt)
        emit_exp(kt)
        if prev is not None:
            emit_pv(prev)
        if kt in fill_at and fq:
            fq.pop(0)()
        prev = kt
    emit_pv(prev)
    for f in fq:
        f()

    # normalize: rank-1 matmul broadcasts 1/den across partitions (no DRAM).
    # Deferred into the next head's stream so the PE never waits on the DVE
    # reciprocal chain at a head boundary.
    nc.vector.reciprocal_approx_fast(out=g.rden, in_=g.den)
    rdenb = g.rd.tile([1, 1024], BF16, tag="rdenb", name=f"rc{h}{half}")
    nc.vector.tensor_copy(out=rdenb, in_=g.rden)

    def finish_norm():
        brps = g.sc.tile([128, 1024], F32, tag="sc", name=f"br{h}_{half}")
        for qb in range(2):
            nc.tensor.matmul(
                brps[:, ts(qb, 512)],
                lhsT=g.ones1,
                rhs=rdenb[0:1, ts(qb, 512)],
                start=True,
                stop=True,
            )
        nc.vector.tensor_tensor(
            out=g.y_sb[jt][hrow : hrow + 64, qlo:qhi],
            in0=g.y_sb[jt][hrow : hrow + 64, qlo:qhi],
            in1=brps[hrow : hrow + 64, :],
            op=MULT,
        )

    return finish_norm


def _trace(nc, tc, io):
    xT, wq, wk, wv, wp, bq, bk, bv, bp, tri, out = io
    g = _Ctx()
    g.nc = nc
    g.out = out

    with (
        tc.tile_pool(name="consts", bufs=1) as consts,
        tc.tile_pool(name="qk", bufs=1) as qk_pool,
        tc.tile_pool(name="vp", bufs=1) as v_pool,
        tc.tile_pool(name="yp", bufs=1) as y_pool,
        tc.tile_pool(name="pt", bufs=3) as pt_pool,
        tc.tile_pool(name="rd", bufs=2) as rd_pool,
        tc.tile_pool(name="osb", bufs=3) as o_pool,
        tc.tile_pool(name="wpp", bufs=1) as wp_pool,
        tc.tile_pool(name="pp", bufs=2, space="PSUM") as pp_pool,
        tc.tile_pool(name="sc", bufs=2, space="PSUM") as sc_pool,
        tc.tile_pool(name="pvp", bufs=1, space="PSUM") as pv_pool,
    ):
        g.pt, g.rd, g.o_pool = pt_pool, rd_pool, o_pool
        g.pp, g.sc, g.pv_pool = pp_pool, sc_pool, pv_pool

        # ---- constants (scalar queue; tiny) -----------------------------
        g.tri_sb = consts.tile([128, 128], BF16, tag="tri")
        nc.scalar.dma_start(out=g.tri_sb, in_=tri)
        g.bq_sb = consts.tile([128, 4], F32, tag="bq")
        nc.scalar.dma_start(out=g.bq_sb, in_=bq.rearrange("(jt p) -> p jt", p=128))
        g.bk_sb = consts.tile([128, 4], F32, tag="bk")
        nc.scalar.dma_start(out=g.bk_sb, in_=bk.rearrange("(jt p) -> p jt", p=128))
        g.bv_sb = consts.tile([128, JC], F32, tag="bv")
        nc.scalar.dma_start(out=g.bv_sb, in_=bv.unsqueeze(0).to_broadcast([128, JC]))
        g.bp_sb = consts.tile([128, C], F32, tag="bp")
        nc.scalar.dma_start(out=g.bp_sb, in_=bp.unsqueeze(0).to_broadcast([128, C]))
        g.ones1 = consts.tile([1, 128], BF16, tag="ones1")
        nc.vector.memset(g.ones1, 1.0)

        g.q_sb = [qk_pool.tile([128, T], BF16, tag=f"q{h}", name=f"q{h}") for h in range(HL)]
        g.k_sb = [qk_pool.tile([128, T], BF16, tag=f"k{jt}", name=f"k{jt}") for jt in range(4)]
        g.v_sb = [v_pool.tile([128, HL, 128], BF16, tag=f"v{tt}", name=f"v{tt}") for tt in range(TT)]
        g.y_sb = [y_pool.tile([128, T], BF16, tag=f"y{jt}", name=f"y{jt}") for jt in range(4)]
        # zero-pad the unused half of each per-head Q tile (full-128 contraction
        # keeps the PE array activity high so the HAM clock gate stays warm)
        for h in range(HL):
            pad = (64, 128) if h % 2 == 0 else (0, 64)
            nc.vector.memset(g.q_sb[h][pad[0] : pad[1], :], 0.0)
        for tt in range(TT):
            nc.vector.memset(g.v_sb[tt], 0.0)
            nc.vector.memset(g.v_sb[tt][:, :, 64:65], 1.0)

        with (
            tc.tile_pool(name="wat", bufs=1) as w_pool,
            tc.tile_pool(name="xt", bufs=2) as xt_pool,
        ):
            # ---- initial DMAs: interleave across both HWDGE queues ------
            # scalar queue: wq, wk slices, wp (done before the first exp).
            # sync queue: xt chunks, wv, then bounces + out stores.
            xT_r = xT.rearrange("(ct p) t -> p ct t", p=128)
            g.wq = w_pool.tile([128, CT, JC], BF16, tag="wq")
            g.wk = w_pool.tile([128, CT, JC], BF16, tag="wk")
            g.wv = w_pool.tile([128, CT, JC], BF16, tag="wv")
            g.xt = [
                xt_pool.tile([128, CT, TCH], BF16, tag="xt", name=f"x{c}")
                for c in range(NCH)
            ]
            nc.scalar.dma_start(out=g.wq, in_=wq.rearrange("(ct p) j -> p ct j", p=128))
            nc.sync.dma_start(out=g.xt[0], in_=xT_r[:, :, ts(0, TCH)])
            nc.scalar.dma_start(out=g.wk, in_=wk.rearrange("(ct p) j -> p ct j", p=128))
            nc.sync.dma_start(out=g.xt[1], in_=xT_r[:, :, ts(1, TCH)])
            nc.scalar.dma_start(out=g.wv, in_=wv.rearrange("(ct p) j -> p ct j", p=128))
            g.wp_sb = wp_pool.tile([128, 4, C], BF16, tag="wp")
            nc.scalar.dma_start(
                out=g.wp_sb, in_=wp.rearrange("(jt p) c -> p jt c", p=128)
            )
            for c in (2, 3):
                nc.sync.dma_start(out=g.xt[c], in_=xT_r[:, :, ts(c, TCH)])

            # ---- phase A: QK then V for t < 1024 ------------------------
            for c in (0, 1):
                for which in ("q", "k"):
                    for jt in range(4):
                        _proj_group(g, which, c, jt)
            for c in (0, 1):
                for sub in range(TCH // 128):
                    _proj_group(g, "v", c, sub)

            # ---- phase B: attention half 0 + projection fillers ---------
            fillers = []
            for c in (2, 3):
                for which in ("q", "k"):
                    for jt in range(4):
                        fillers.append(
                            (lambda wh=which, cc=c, j=jt: _proj_group(g, wh, cc, j))
                        )
            for c in (2, 3):
                for sub in range(TCH // 128):
                    fillers.append((lambda cc=c, s=sub: _proj_group(g, "v", cc, s)))
            per = (len(fillers) + HL - 1) // HL
            pending = []
            for h in range(HL):
                fl = pending + fillers[h * per : (h + 1) * per]
                pending = [_attn_half(g, h, 0, fl)]

        # ---- phase C: attention half 1 + out-proj t<1024 fillers --------
        for h in range(HL):
            fl = pending + [lambda t0=h: _oproj_tile(g, t0)]
            pending = [_attn_half(g, h, 1, fl)]
        for f in pending:
            f()

        # ---- phase D: out-proj t >= 1024 --------------------------------
        for tt in range(8, 16):
            _oproj_tile(g, tt)


_CACHE = {}


def build_nc():
    if "nc" in _CACHE:
        return _CACHE["nc"]
    nc = bacc.Bacc(
        "TRN2",
        target_bir_lowering=False,
        debug=False,
        enable_asserts=False,
        num_devices=NCORES,
    )
    io = (
        nc.dram_tensor("xT", [C, T], BF16, kind="ExternalInput").ap(),
        nc.dram_tensor("wq", [C, JC], BF16, kind="ExternalInput").ap(),
        nc.dram_tensor("wk", [C, JC], BF16, kind="ExternalInput").ap(),
        nc.dram_tensor("wv", [C, JC], BF16, kind="ExternalInput").ap(),
        nc.dram_tensor("wp", [JC, C], BF16, kind="ExternalInput").ap(),
        nc.dram_tensor("bq", [JC], F32, kind="ExternalInput").ap(),
        nc.dram_tensor("bk", [JC], F32, kind="ExternalInput").ap(),
        nc.dram_tensor("bv", [JC], F32, kind="ExternalInput").ap(),
        nc.dram_tensor("bp", [C], F32, kind="ExternalInput").ap(),
        nc.dram_tensor("tri", [128, 128], BF16, kind="ExternalInput").ap(),
        nc.dram_tensor("out", [T, C], F32, kind="ExternalOutput").ap(),
    )
    with tile.TileContext(nc) as tc:
        _trace(nc, tc, io)
    nc.compile()
    _CACHE["nc"] = nc
    return nc


def make_in_maps(x, w_attn, b_attn, w_proj, b_proj):
    import ml_dtypes
    tri = np.triu(np.ones((128, 128), dtype=ml_dtypes.bfloat16))
    zeros_c = np.zeros(C, dtype=np.float32)
    in_maps = []
    for core in range(NCORES):
        b, hh = core // 2, core % 2
        j0 = JC * hh
        in_maps.append(
            {
                "xT": np.ascontiguousarray(x[b].T).astype(ml_dtypes.bfloat16),
                "wq": np.ascontiguousarray(w_attn[:, j0 : j0 + JC]).astype(ml_dtypes.bfloat16),
                "wk": np.ascontiguousarray(w_attn[:, C + j0 : C + j0 + JC]).astype(ml_dtypes.bfloat16),
                "wv": np.ascontiguousarray(w_attn[:, 2 * C + j0 : 2 * C + j0 + JC]).astype(ml_dtypes.bfloat16),
                "wp": np.ascontiguousarray(w_proj[j0 : j0 + JC, :]).astype(ml_dtypes.bfloat16),
                "bq": np.ascontiguousarray(b_attn[j0 : j0 + JC]),
                "bk": np.ascontiguousarray(b_attn[C + j0 : C + j0 + JC]),
                "bv": np.ascontiguousarray(b_attn[2 * C + j0 : 2 * C + j0 + JC]),
                "bp": (b_proj.astype(np.float32) if hh == 0 else zeros_c),
                "tri": tri,
            }
        )
    return in_maps


def gather(parts):
    out = np.empty((B, T, C), dtype=np.float32)
    for b in range(B):
        out[b] = parts[2 * b]["out"] + parts[2 * b + 1]["out"]
    return out


def kernel(x, w_attn, b_attn, w_proj, b_proj):
    x = np.asarray(x, dtype=np.float32)
    w_attn = np.asarray(w_attn, dtype=np.float32)
    b_attn = np.asarray(b_attn, dtype=np.float32)
    w_proj = np.asarray(w_proj, dtype=np.float32)
    b_proj = np.asarray(b_proj, dtype=np.float32)
    nc = build_nc()
    in_maps = make_in_maps(x, w_attn, b_attn, w_proj, b_proj)
    res = run_bass_kernel_spmd(nc, in_maps, core_ids=list(range(NCORES)))
    return gather(res.results)


if __name__ == "__main__":
    rng = np.random.default_rng(0)
    x = rng.standard_normal((B, T, C), dtype=np.float32)
    w_attn = rng.standard_normal((C, 3 * C), dtype=np.float32) / np.sqrt(C)
    b_attn = np.zeros(3 * C, np.float32)
    w_proj = rng.standard_normal((C, C), dtype=np.float32) / np.sqrt(C)
    b_proj = np.zeros(C, np.float32)
    out = kernel(x, w_attn, b_attn, w_proj, b_proj)
    print(out.shape, out.dtype, np.abs(out).mean())


# revision 16
# speedup vs baseline: 1.1749x; 1.0006x over previous
"""Trainium2 Bass kernel for causal multi-head attention (B=4, T=2048, C=1024, H=16).

Sharding: tensor-parallel over heads x batch. 8 cores = 4 batches x 2 head-halves.
Each core computes, for its batch b and its 8 heads:
  qkv projection -> causal attention -> output projection partial (rows of w_proj)
Host gathers by summing the two half-partials per batch (the "all-reduce").

Schedule (v2): single fused instruction stream keeps the PE dense so the HAM
clock gate stays warm (2.4 GHz) and the scalar engine (exp) is fed from ~30us:
  A: Q,K projections for t<1024, then V for t<1024.
  B: per head, attention for q<1024 (scores pipelined one kt ahead of exp->PV),
     interleaved with Q,K,V projection groups for t>=1024 as PE filler.
  C: per head, attention for q>=1024, interleaved with out-proj tiles t<1024.
  D: out-proj tiles t>=1024.
Initial DMAs are split per-128-row slice across both HWDGE queues (sync + act)
so the first matmul starts ~2us after the preamble instead of ~23us.

Per-core layouts / precision:
  x slices  [128, 512] f32r    contraction dim c on partitions.
  Q^T, K^T as [j=512, T] bf16  (4 partition-tiles of 2 heads each). K is
  pre-scaled by log2(e)/8 so scores S' = S*log2(e)/8 and exp via 2^{S'}
  (scalar activation Exp with scale=ln2). Scores are computed transposed:
  S'^T[k, q] = sum_d K'^T[d,k] Q^T[d,q], so P^T feeds the PV matmul directly.
  V as [t, h, 65] bf16 with a ones column per head: row 64 of the PV output is
  the softmax denominator; the division happens after an unnormalized copy of
  the PV psum into y (frees the psum bank early), via a DRAM-bounce broadcast.
  Causal masks (diagonal 128x128 blocks) multiply on the gpsimd engine.
  QKV projections and the output projection contract in fp32r; attention
  matmuls run in bf16; all PSUM accumulation is fp32.
"""

import sys

for _p in ("/opt/trn_rl_repo",):
    if _p not in sys.path:
        sys.path.insert(0, _p)

import math

import numpy as np

import concourse.bass as bass
import concourse.mybir as mybir
import concourse.tile as tile
from concourse import bacc
from concourse.bass import ts
from concourse.bass_utils import run_bass_kernel_spmd

B, T, C, H, D = 4, 2048, 1024, 16, 64
NCORES = 8
JC = 512  # channels per core (8 heads x 64)
HL = 8  # heads per core
CT = C // 128  # 8 contraction tiles
TT = T // 128  # 16 t(=k) tiles
TCH = 512  # projection t-chunk
NCH = T // TCH  # 4 chunks
F32 = mybir.dt.float32
F32R = mybir.dt.float32r
BF16 = mybir.dt.bfloat16
EXP = mybir.ActivationFunctionType.Exp
ADD = mybir.AluOpType.add
MULT = mybir.AluOpType.mult
KSCALE = math.log2(math.e) / 8.0  # folded into K so probs = 2^{S'}
LN2 = math.log(2.0)


def _r(ap):
    return ap.bitcast(F32R)


class _Ctx:
    pass


def _proj_group(g, which, c, jt):
    """One projection psum group: 8 accumulating matmuls + drain.

    which in {q, k, v}; c = t-chunk; jt = output 128-slice (for q/k) or
    t-sub-block (for v).
    """
    nc = g.nc
    if which == "v":
        # out tile [128 t, JC j] for t-sub-block jt of chunk c
        tt = c * (TCH // 128) + jt
        ps = g.pp.tile([128, JC], F32, tag="pp", name=f"pv_ps{tt}")
        for ct in range(CT):
            nc.tensor.matmul(
                ps,
                lhsT=g.xt[c][:, ct, ts(jt, 128)],
                rhs=g.wv[:, ct, :],
                start=(ct == 0),
                stop=(ct == CT - 1),
            )
        nc.vector.tensor_tensor(
            out=g.v_sb[tt][:, :, 0:64],
            in0=ps.rearrange("p (h d) -> p h d", h=HL),
            in1=g.bv_sb.rearrange("p (h d) -> p h d", h=HL),
            op=ADD,
        )
    else:
        w, dst, bcol = (g.wq, g.q_sb, g.bq_sb) if which == "q" else (g.wk, g.k_sb, g.bk_sb)
        ps = g.pp.tile([128, TCH], F32, tag="pp", name=f"p{which}{c}_{jt}")
        for ct in range(CT):
            nc.tensor.matmul(
                ps,
                lhsT=w[:, ct, ts(jt, 128)],
                rhs=g.xt[c][:, ct, :],
                start=(ct == 0),
                stop=(ct == CT - 1),
            )
        if which == "q":
            for piece in range(2):
                rows = slice(64 * piece, 64 * piece + 64)
                nc.vector.tensor_scalar_add(
                    out=dst[2 * jt + piece][rows, ts(c, TCH)],
                    in0=ps[rows, :],
                    scalar1=bcol[rows, jt : jt + 1],
                )
        else:
            nc.vector.tensor_scalar(
                out=dst[jt][:, ts(c, TCH)],
                in0=ps,
                scalar1=bcol[:, jt : jt + 1],
                scalar2=KSCALE,
                op0=ADD,
                op1=MULT,
            )


def _oproj_tile(g, tt):
    """Output projection for t-block tt: y[:, tt] @ wp + bp -> out DMA."""
    nc = g.nc
    ot = g.o_pool.tile([128, C], F32, tag="o", name=f"ot{tt}")
    for ch in range(2):
        ps = g.pp.tile([128, 512], F32, tag="pp", name=f"op{tt}_{ch}")
        for jt in range(4):
            nc.tensor.matmul(
                ps,
                lhsT=g.y_sb[jt][:, ts(tt, 128)],
                rhs=g.wp_sb[:, jt, ts(ch, 512)],
                start=(jt == 0),
                stop=(jt == 3),
            )
        nc.vector.tensor_tensor(
            out=ot[:, ts(ch, 512)], in0=ps, in1=g.bp_sb[:, ts(ch, 512)], op=ADD
        )
    nc.sync.dma_start(out=g.out[ts(tt, 128), :], in_=ot)


def _attn_half(g, h, half, fillers):
    """Causal attention for head h, query range [1024*half, 1024*(half+1)).

    Scores for kt+1 are emitted before PV for kt so the PE never waits on the
    scalar engine's exp. `fillers` is a list of zero-arg emitters (projection /
    out-proj groups) drained at fixed points to fill PE time while exp runs.
    """
    nc = g.nc
    jt, hrow = h // 2, 64 * (h % 2)
    qlo, qhi = 1024 * half, 1024 * (half + 1)
    nkt = qhi // 128
    fill_at = {2, 5, 9, 13}

    sc_t = {}
    pt_t = {}

    def emit_scores(kt):
        qstart = max(qlo, 128 * kt)
        w = qhi - qstart
        scps = g.sc.tile([128, 1024], F32, tag="sc", name=f"sc{h}_{half}_{kt}")
        sc_t[kt] = (scps, qstart, w)
        for q5 in range(0, w, 512):
            w5 = min(512, w - q5)
            nc.tensor.matmul(
                scps[:, q5 : q5 + w5],
                lhsT=g.k_sb[jt][:, ts(kt, 128)],
                rhs=g.q_sb[h][:, qstart + q5 : qstart + q5 + w5],
                start=True,
                stop=True,
            )

    def emit_exp(kt):
        scps, qstart, w = sc_t[kt]
        pt = g.pt.tile([128, 1024], BF16, tag="pt", name=f"pt{h}_{half}_{kt}")
        pt_t[kt] = pt
        nc.scalar.activation(out=pt[:, 0:w], in_=scps[:, 0:w], func=EXP, scale=LN2)
        if qstart == 128 * kt:
            nc.gpsimd.tensor_tensor(
                out=pt[:, 0:128], in0=pt[:, 0:128], in1=g.tri_sb, op=MULT
            )

    def emit_pv(kt):
        _, qstart, _ = sc_t[kt]
        pt = pt_t.pop(kt)
        for qb in range(2):
            blo = qlo + 512 * qb
            bhi = blo + 512
            lo = max(qstart, blo)
            if lo >= bhi:
                continue
            last_kt = bhi // 128 - 1
            nc.tensor.matmul(
                g.pv[qb][:, lo - blo : 512],
                lhsT=g.v_sb[kt][:, h, :],
                rhs=pt[:, lo - qstart : bhi - qstart],
                start=(kt == 0),
                stop=(kt == last_kt),
            )
            if kt == last_kt:
                # unnormalized drain frees the psum bank for the next head
                nc.vector.tensor_copy(
                    out=g.y_sb[jt][hrow : hrow + 64, blo:bhi], in_=g.pv[qb][0:64, :]
                )
                nc.vector.tensor_copy(
                    out=g.den[0:1, ts(qb, 512)], in_=g.pv[qb][64:65, :]
                )

    g.pv = [
        g.pv_pool.tile([128, 512], F32, tag=f"pv{qb}", name=f"pv{h}_{half}_{qb}")
        for qb in range(2)
    ]  # bufs=1 per tag: 2 PSUM banks total; early unnormalized drain frees them
    g.den = g.rd.tile([1, 1024], F32, tag="den", name=f"dn{h}{half}")
    g.rden = g.rd.tile([1, 1024], F32, tag="rden", name=f"rr{h}{half}")

    fq = list(fillers)
    prev = None
    for kt in range(nkt):
        emit_scores(kt)
        emit_exp(kt)
        if prev is not None:
            emit_pv(prev)
        if kt in fill_at and fq:
            fq.pop(0)()
        prev = kt
    emit_pv(prev)
    for f in fq:
        f()

    # normalize: rank-1 matmul broadcasts 1/den across partitions (no DRAM).
    # Deferred into the next head's stream so the PE never waits on the DVE
    # reciprocal chain at a head boundary.
    nc.vector.reciprocal_approx_fast(out=g.rden, in_=g.den)
    rdenb = g.rd.tile([1, 1024], BF16, tag="rdenb", name=f"rc{h}{half}")
    nc.vector.tensor_copy(out=rdenb, in_=g.rden)

    def finish_norm():
        brps = g.sc.tile([128, 1024], F32, tag="sc", name=f"br{h}_{half}")
        for qb in range(2):
            nc.tensor.matmul(
                brps[:, ts(qb, 512)],
                lhsT=g.ones1,
                rhs=rdenb[0:1, ts(qb, 512)],
                start=True,
                stop=True,
            )
        nc.vector.tensor_tensor(
            out=g.y_sb[jt][hrow : hrow + 64, qlo:qhi],
            in0=g.y_sb[jt][hrow : hrow + 64, qlo:qhi],
            in1=brps[hrow : hrow + 64, :],
            op=MULT,
        )

    return finish_norm


def _trace(nc, tc, io):
    xT, wq, wk, wv, wp, cpk, tri, out = io
    g = _Ctx()
    g.nc = nc
    g.out = out

    with (
        tc.tile_pool(name="consts", bufs=1) as consts,
        tc.tile_pool(name="qk", bufs=1) as qk_pool,
        tc.tile_pool(name="vp", bufs=1) as v_pool,
        tc.tile_pool(name="yp", bufs=1) as y_pool,
        tc.tile_pool(name="pt", bufs=3) as pt_pool,
        tc.tile_pool(name="rd", bufs=2) as rd_pool,
        tc.tile_pool(name="osb", bufs=3) as o_pool,
        tc.tile_pool(name="wpp", bufs=1) as wp_pool,
        tc.tile_pool(name="pp", bufs=2, space="PSUM") as pp_pool,
        tc.tile_pool(name="sc", bufs=2, space="PSUM") as sc_pool,
        tc.tile_pool(name="pvp", bufs=1, space="PSUM") as pv_pool,
    ):
        g.pt, g.rd, g.o_pool = pt_pool, rd_pool, o_pool
        g.pp, g.sc, g.pv_pool = pp_pool, sc_pool, pv_pool

        # ---- constants: one packed DMA (bq|bk|bv|bp) + tri --------------
        g.cpk = consts.tile([128, 4 + 4 + JC + C], F32, tag="cpk")
        g.bq_sb = g.cpk[:, 0:4]
        g.bk_sb = g.cpk[:, 4:8]
        g.bv_sb = g.cpk[:, 8 : 8 + JC]
        g.bp_sb = g.cpk[:, 8 + JC : 8 + JC + C]
        g.tri_sb = consts.tile([128, 128], BF16, tag="tri")
        g.ones1 = consts.tile([1, 128], BF16, tag="ones1")
        nc.vector.memset(g.ones1, 1.0)

        g.q_sb = [qk_pool.tile([128, T], BF16, tag=f"q{h}", name=f"q{h}") for h in range(HL)]
        g.k_sb = [qk_pool.tile([128, T], BF16, tag=f"k{jt}", name=f"k{jt}") for jt in range(4)]
        g.v_sb = [v_pool.tile([128, HL, 128], BF16, tag=f"v{tt}", name=f"v{tt}") for tt in range(TT)]
        g.y_sb = [y_pool.tile([128, T], BF16, tag=f"y{jt}", name=f"y{jt}") for jt in range(4)]
        # zero-pad the unused half of each per-head Q tile (full-128 contraction
        # keeps the PE array activity high so the HAM clock gate stays warm)
        for h in range(HL):
            pad = (64, 128) if h % 2 == 0 else (0, 64)
            nc.vector.memset(g.q_sb[h][pad[0] : pad[1], :], 0.0)
        for tt in range(TT):
            nc.vector.memset(g.v_sb[tt], 0.0)
            nc.vector.memset(g.v_sb[tt][:, :, 64:65], 1.0)

        with (
            tc.tile_pool(name="wat", bufs=1) as w_pool,
            tc.tile_pool(name="xt", bufs=2) as xt_pool,
        ):
            # ---- initial DMAs: interleave across both HWDGE queues ------
            # scalar queue: wq, wk slices, wp (done before the first exp).
            # sync queue: xt chunks, wv, then bounces + out stores.
            xT_r = xT.rearrange("(ct p) t -> p ct t", p=128)
            g.wq = w_pool.tile([128, CT, JC], BF16, tag="wq")
            g.wk = w_pool.tile([128, CT, JC], BF16, tag="wk")
            g.wv = w_pool.tile([128, CT, JC], BF16, tag="wv")
            g.xt = [
                xt_pool.tile([128, CT, TCH], BF16, tag="xt", name=f"x{c}")
                for c in range(NCH)
            ]
            nc.scalar.dma_start(out=g.wq, in_=wq.rearrange("(ct p) j -> p ct j", p=128))
            nc.scalar.dma_start(out=g.cpk, in_=cpk)
            nc.scalar.dma_start(out=g.tri_sb, in_=tri)
            nc.sync.dma_start(out=g.xt[0], in_=xT_r[:, :, ts(0, TCH)])
            nc.scalar.dma_start(out=g.wk, in_=wk.rearrange("(ct p) j -> p ct j", p=128))
            nc.sync.dma_start(out=g.xt[1], in_=xT_r[:, :, ts(1, TCH)])
            nc.scalar.dma_start(out=g.wv, in_=wv.rearrange("(ct p) j -> p ct j", p=128))
            g.wp_sb = wp_pool.tile([128, 4, C], BF16, tag="wp")
            nc.scalar.dma_start(
                out=g.wp_sb, in_=wp.rearrange("(jt p) c -> p jt c", p=128)
            )
            for c in (2, 3):
                nc.sync.dma_start(out=g.xt[c], in_=xT_r[:, :, ts(c, TCH)])

            # ---- phase A: QK then V for t < 1024 ------------------------
            for c in (0, 1):
                for which in ("q", "k"):
                    for jt in range(4):
                        _proj_group(g, which, c, jt)
            for c in (0, 1):
                for sub in range(TCH // 128):
                    _proj_group(g, "v", c, sub)

            # ---- phase B: attention half 0 + projection fillers ---------
            fillers = []
            for c in (2, 3):
                for which in ("q", "k"):
                    for jt in range(4):
                        fillers.append(
                            (lambda wh=which, cc=c, j=jt: _proj_group(g, wh, cc, j))
                        )
            for c in (2, 3):
                for sub in range(TCH // 128):
                    fillers.append((lambda cc=c, s=sub: _proj_group(g, "v", cc, s)))
            per = (len(fillers) + HL - 1) // HL
            pending = []
            for h in range(HL):
                fl = pending + fillers[h * per : (h + 1) * per]
                pending = [_attn_half(g, h, 0, fl)]

        # ---- phase C: attention half 1 + out-proj t<1024 fillers --------
        for h in range(HL):
            fl = pending + [lambda t0=h: _oproj_tile(g, t0)]
            pending = [_attn_half(g, h, 1, fl)]
        for f in pending:
            f()

        # ---- phase D: out-proj t >= 1024 --------------------------------
        for tt in range(8, 16):
            _oproj_tile(g, tt)


_CACHE = {}


def build_nc():
    if "nc" in _CACHE:
        return _CACHE["nc"]
    nc = bacc.Bacc(
        "TRN2",
        target_bir_lowering=False,
        debug=False,
        enable_asserts=False,
        num_devices=NCORES,
    )
    io = (
        nc.dram_tensor("xT", [C, T], BF16, kind="ExternalInput").ap(),
        nc.dram_tensor("wq", [C, JC], BF16, kind="ExternalInput").ap(),
        nc.dram_tensor("wk", [C, JC], BF16, kind="ExternalInput").ap(),
        nc.dram_tensor("wv", [C, JC], BF16, kind="ExternalInput").ap(),
        nc.dram_tensor("wp", [JC, C], BF16, kind="ExternalInput").ap(),
        nc.dram_tensor("cpk", [128, 4 + 4 + JC + C], F32, kind="ExternalInput").ap(),
        nc.dram_tensor("tri", [128, 128], BF16, kind="ExternalInput").ap(),
        nc.dram_tensor("out", [T, C], F32, kind="ExternalOutput").ap(),
    )
    with tile.TileContext(nc) as tc:
        _trace(nc, tc, io)
    nc.compile()
    _CACHE["nc"] = nc
    return nc


def make_in_maps(x, w_attn, b_attn, w_proj, b_proj):
    import ml_dtypes
    tri = np.triu(np.ones((128, 128), dtype=ml_dtypes.bfloat16))
    zeros_c = np.zeros(C, dtype=np.float32)
    in_maps = []
    for core in range(NCORES):
        b, hh = core // 2, core % 2
        j0 = JC * hh
        bq = b_attn[j0 : j0 + JC].astype(np.float32)
        bk = b_attn[C + j0 : C + j0 + JC].astype(np.float32)
        bv = b_attn[2 * C + j0 : 2 * C + j0 + JC].astype(np.float32)
        bp = b_proj.astype(np.float32) if hh == 0 else zeros_c
        cpk = np.empty((128, 4 + 4 + JC + C), dtype=np.float32)
        cpk[:, 0:4] = bq.reshape(4, 128).T
        cpk[:, 4:8] = bk.reshape(4, 128).T
        cpk[:, 8 : 8 + JC] = bv[None, :]
        cpk[:, 8 + JC :] = bp[None, :]
        in_maps.append(
            {
                "xT": np.ascontiguousarray(x[b].T).astype(ml_dtypes.bfloat16),
                "wq": np.ascontiguousarray(w_attn[:, j0 : j0 + JC]).astype(ml_dtypes.bfloat16),
                "wk": np.ascontiguousarray(w_attn[:, C + j0 : C + j0 + JC]).astype(ml_dtypes.bfloat16),
                "wv": np.ascontiguousarray(w_attn[:, 2 * C + j0 : 2 * C + j0 + JC]).astype(ml_dtypes.bfloat16),
                "wp": np.ascontiguousarray(w_proj[j0 : j0 + JC, :]).astype(ml_dtypes.bfloat16),
                "cpk": cpk,
                "tri": tri,
            }
        )
    return in_maps


def gather(parts):
    out = np.empty((B, T, C), dtype=np.float32)
    for b in range(B):
        out[b] = parts[2 * b]["out"] + parts[2 * b + 1]["out"]
    return out


def kernel(x, w_attn, b_attn, w_proj, b_proj):
    x = np.asarray(x, dtype=np.float32)
    w_attn = np.asarray(w_attn, dtype=np.float32)
    b_attn = np.asarray(b_attn, dtype=np.float32)
    w_proj = np.asarray(w_proj, dtype=np.float32)
    b_proj = np.asarray(b_proj, dtype=np.float32)
    nc = build_nc()
    in_maps = make_in_maps(x, w_attn, b_attn, w_proj, b_proj)
    res = run_bass_kernel_spmd(nc, in_maps, core_ids=list(range(NCORES)))
    return gather(res.results)


if __name__ == "__main__":
    rng = np.random.default_rng(0)
    x = rng.standard_normal((B, T, C), dtype=np.float32)
    w_attn = rng.standard_normal((C, 3 * C), dtype=np.float32) / np.sqrt(C)
    b_attn = np.zeros(3 * C, np.float32)
    w_proj = rng.standard_normal((C, C), dtype=np.float32) / np.sqrt(C)
    b_proj = np.zeros(C, np.float32)
    out = kernel(x, w_attn, b_attn, w_proj, b_proj)
    print(out.shape, out.dtype, np.abs(out).mean())


# revision 17
# speedup vs baseline: 1.2399x; 1.0553x over previous
"""Trainium2 Bass kernel for causal multi-head attention (B=4, T=2048, C=1024, H=16).

Sharding: tensor-parallel over heads x batch. 8 cores = 4 batches x 2 head-halves.
Each core computes, for its batch b and its 8 heads:
  qkv projection -> causal attention -> output projection partial (rows of w_proj)
Host gathers by summing the two half-partials per batch (the "all-reduce").

Schedule (v2): single fused instruction stream keeps the PE dense so the HAM
clock gate stays warm (2.4 GHz) and the scalar engine (exp) is fed from ~30us:
  A: Q,K projections for t<1024, then V for t<1024.
  B: per head, attention for q<1024 (scores pipelined one kt ahead of exp->PV),
     interleaved with Q,K,V projection groups for t>=1024 as PE filler.
  C: per head, attention for q>=1024, interleaved with out-proj tiles t<1024.
  D: out-proj tiles t>=1024.
Initial DMAs are split per-128-row slice across both HWDGE queues (sync + act)
so the first matmul starts ~2us after the preamble instead of ~23us.

Per-core layouts / precision:
  x slices  [128, 512] f32r    contraction dim c on partitions.
  Q^T, K^T as [j=512, T] bf16  (4 partition-tiles of 2 heads each). K is
  pre-scaled by log2(e)/8 so scores S' = S*log2(e)/8 and exp via 2^{S'}
  (scalar activation Exp with scale=ln2). Scores are computed transposed:
  S'^T[k, q] = sum_d K'^T[d,k] Q^T[d,q], so P^T feeds the PV matmul directly.
  V as [t, h, 65] bf16 with a ones column per head: row 64 of the PV output is
  the softmax denominator; the division happens after an unnormalized copy of
  the PV psum into y (frees the psum bank early), via a DRAM-bounce broadcast.
  Causal masks (diagonal 128x128 blocks) multiply on the gpsimd engine.
  QKV projections and the output projection contract in fp32r; attention
  matmuls run in bf16; all PSUM accumulation is fp32.
"""

import sys

for _p in ("/opt/trn_rl_repo",):
    if _p not in sys.path:
        sys.path.insert(0, _p)

import math

import numpy as np

import concourse.bass as bass
import concourse.mybir as mybir
import concourse.tile as tile
from concourse import bacc
from concourse.bass import ts
from concourse.bass_utils import run_bass_kernel_spmd

B, T, C, H, D = 4, 2048, 1024, 16, 64
NCORES = 8
JC = 512  # channels per core (8 heads x 64)
HL = 8  # heads per core
CT = C // 128  # 8 contraction tiles
TT = T // 128  # 16 t(=k) tiles
TCH = 512  # projection t-chunk
NCH = T // TCH  # 4 chunks
F32 = mybir.dt.float32
F32R = mybir.dt.float32r
BF16 = mybir.dt.bfloat16
EXP = mybir.ActivationFunctionType.Exp
ADD = mybir.AluOpType.add
MULT = mybir.AluOpType.mult
KSCALE = math.log2(math.e) / 8.0  # folded into K so probs = 2^{S'}
LN2 = math.log(2.0)


def _r(ap):
    return ap.bitcast(F32R)


class _Ctx:
    pass


def _proj_group(g, which, c, jt):
    """One projection psum group: 8 accumulating matmuls + drain.

    which in {q, k, v}; c = t-chunk; jt = output 128-slice (for q/k) or
    t-sub-block (for v).
    """
    nc = g.nc
    if which == "v":
        # out tile [128 t, JC j] for t-sub-block jt of chunk c
        tt = c * (TCH // 128) + jt
        ps = g.pp.tile([128, JC], F32, tag="pp", name=f"pv_ps{tt}")
        for ct in range(CT):
            nc.tensor.matmul(
                ps,
                lhsT=g.xt[c][:, ct, ts(jt, 128)],
                rhs=g.wv[:, ct, :],
                start=(ct == 0),
                stop=(ct == CT - 1),
            )
        nc.vector.tensor_tensor(
            out=g.v_sb[tt][:, :, 0:64],
            in0=ps.rearrange("p (h d) -> p h d", h=HL),
            in1=g.bv_sb.rearrange("p (h d) -> p h d", h=HL),
            op=ADD,
        )
    else:
        w, dst, bcol = (g.wq, g.q_sb, g.bq_sb) if which == "q" else (g.wk, g.k_sb, g.bk_sb)
        ps = g.pp.tile([128, TCH], F32, tag="pp", name=f"p{which}{c}_{jt}")
        for ct in range(CT):
            nc.tensor.matmul(
                ps,
                lhsT=w[:, ct, ts(jt, 128)],
                rhs=g.xt[c][:, ct, :],
                start=(ct == 0),
                stop=(ct == CT - 1),
            )
        if which == "q":
            for piece in range(2):
                rows = slice(64 * piece, 64 * piece + 64)
                nc.vector.tensor_scalar_add(
                    out=dst[2 * jt + piece][rows, ts(c, TCH)],
                    in0=ps[rows, :],
                    scalar1=bcol[rows, jt : jt + 1],
                )
        else:
            nc.vector.tensor_scalar(
                out=dst[jt][:, ts(c, TCH)],
                in0=ps,
                scalar1=bcol[:, jt : jt + 1],
                scalar2=KSCALE,
                op0=ADD,
                op1=MULT,
            )


def _oproj_tile(g, tt):
    """Output projection for t-block tt: y[:, tt] @ wp + bp -> out DMA."""
    nc = g.nc
    ot = g.o_pool.tile([128, C], F32, tag="o", name=f"ot{tt}")
    for ch in range(2):
        ps = g.pp.tile([128, 512], F32, tag="pp", name=f"op{tt}_{ch}")
        for jt in range(4):
            nc.tensor.matmul(
                ps,
                lhsT=g.y_sb[jt][:, ts(tt, 128)],
                rhs=g.wp_sb[:, jt, ts(ch, 512)],
                start=(jt == 0),
                stop=(jt == 3),
            )
        nc.vector.tensor_tensor(
            out=ot[:, ts(ch, 512)], in0=ps, in1=g.bp_sb[:, ts(ch, 512)], op=ADD
        )
    nc.sync.dma_start(out=g.out[ts(tt, 128), :], in_=ot)


def _attn_half(g, h, half, fillers):
    """Causal attention for head h, query range [1024*half, 1024*(half+1)).

    Scores for kt+1 are emitted before PV for kt so the PE never waits on the
    scalar engine's exp. `fillers` is a list of zero-arg emitters (projection /
    out-proj groups) drained at fixed points to fill PE time while exp runs.
    """
    nc = g.nc
    jt, hrow = h // 2, 64 * (h % 2)
    qlo, qhi = 1024 * half, 1024 * (half + 1)
    nkt = qhi // 128
    fill_at = {2, 5, 9, 13}

    sc_t = {}
    pt_t = {}

    def emit_scores(kt):
        qstart = max(qlo, 128 * kt)
        w = qhi - qstart
        scps = g.sc.tile([128, 1024], F32, tag="sc", name=f"sc{h}_{half}_{kt}")
        sc_t[kt] = (scps, qstart, w)
        for q5 in range(0, w, 512):
            w5 = min(512, w - q5)
            nc.tensor.matmul(
                scps[:, q5 : q5 + w5],
                lhsT=g.k_sb[jt][:, ts(kt, 128)],
                rhs=g.q_sb[h][:, qstart + q5 : qstart + q5 + w5],
                start=True,
                stop=True,
            )

    def emit_exp(kt):
        scps, qstart, w = sc_t[kt]
        pt = g.pt.tile([128, 1024], BF16, tag="pt", name=f"pt{h}_{half}_{kt}")
        pt_t[kt] = pt
        nc.scalar.activation(out=pt[:, 0:w], in_=scps[:, 0:w], func=EXP, scale=LN2)
        if qstart == 128 * kt:
            nc.gpsimd.tensor_tensor(
                out=pt[:, 0:128], in0=pt[:, 0:128], in1=g.tri_sb, op=MULT
            )

    def emit_pv(kt):
        _, qstart, _ = sc_t[kt]
        pt = pt_t.pop(kt)
        for qb in range(2):
            blo = qlo + 512 * qb
            bhi = blo + 512
            lo = max(qstart, blo)
            if lo >= bhi:
                continue
            last_kt = bhi // 128 - 1
            nc.tensor.matmul(
                g.pv[qb][:, lo - blo : 512],
                lhsT=g.v_sb[kt][:, h, :],
                rhs=pt[:, lo - qstart : bhi - qstart],
                start=(kt == 0),
                stop=(kt == last_kt),
            )
            if kt == last_kt:
                # unnormalized drain frees the psum bank for the next head
                nc.vector.tensor_copy(
                    out=g.y_sb[jt][hrow : hrow + 64, blo:bhi], in_=g.pv[qb][0:64, :]
                )
                nc.vector.tensor_copy(
                    out=g.den[0:1, ts(qb, 512)], in_=g.pv[qb][64:65, :]
                )

    g.pv = [
        g.pv_pool.tile([128, 512], F32, tag=f"pv{qb}", name=f"pv{h}_{half}_{qb}")
        for qb in range(2)
    ]  # bufs=1 per tag: 2 PSUM banks total; early unnormalized drain frees them
    g.den = g.rd.tile([1, 1024], F32, tag="den", name=f"dn{h}{half}")
    g.rden = g.rd.tile([1, 1024], F32, tag="rden", name=f"rr{h}{half}")

    fq = list(fillers)
    prev = None
    for kt in range(nkt):
        emit_scores(kt)
        emit_exp(kt)
        if prev is not None:
            emit_pv(prev)
        if kt in fill_at and fq:
            fq.pop(0)()
        prev = kt
    emit_pv(prev)
    for f in fq:
        f()

    # normalize: rank-1 matmul broadcasts 1/den across partitions (no DRAM).
    # Deferred into the next head's stream so the PE never waits on the DVE
    # reciprocal chain at a head boundary.
    nc.vector.reciprocal_approx_fast(out=g.rden, in_=g.den)
    rdenb = g.rd.tile([1, 1024], BF16, tag="rdenb", name=f"rc{h}{half}")
    nc.vector.tensor_copy(out=rdenb, in_=g.rden)

    def finish_norm():
        brps = g.sc.tile([128, 1024], F32, tag="sc", name=f"br{h}_{half}")
        for qb in range(2):
            nc.tensor.matmul(
                brps[:, ts(qb, 512)],
                lhsT=g.ones1,
                rhs=rdenb[0:1, ts(qb, 512)],
                start=True,
                stop=True,
            )
        nc.vector.tensor_tensor(
            out=g.y_sb[jt][hrow : hrow + 64, qlo:qhi],
            in0=g.y_sb[jt][hrow : hrow + 64, qlo:qhi],
            in1=brps[hrow : hrow + 64, :],
            op=MULT,
        )

    return finish_norm


def _trace(nc, tc, io):
    xT, wq, wk, wv, wp, cpk, tri, out = io
    g = _Ctx()
    g.nc = nc
    g.out = out

    with (
        tc.tile_pool(name="consts", bufs=1) as consts,
        tc.tile_pool(name="qk", bufs=1) as qk_pool,
        tc.tile_pool(name="vp", bufs=1) as v_pool,
        tc.tile_pool(name="yp", bufs=1) as y_pool,
        tc.tile_pool(name="pt", bufs=3) as pt_pool,
        tc.tile_pool(name="rd", bufs=2) as rd_pool,
        tc.tile_pool(name="osb", bufs=3) as o_pool,
        tc.tile_pool(name="wpp", bufs=1) as wp_pool,
        tc.tile_pool(name="pp", bufs=2, space="PSUM") as pp_pool,
        tc.tile_pool(name="sc", bufs=2, space="PSUM") as sc_pool,
        tc.tile_pool(name="pvp", bufs=1, space="PSUM") as pv_pool,
    ):
        g.pt, g.rd, g.o_pool = pt_pool, rd_pool, o_pool
        g.pp, g.sc, g.pv_pool = pp_pool, sc_pool, pv_pool

        # ---- constants: one packed DMA (bq|bk|bv|bp) + tri --------------
        g.cpk = consts.tile([128, 4 + 4 + JC + C], F32, tag="cpk")
        g.bq_sb = g.cpk[:, 0:4]
        g.bk_sb = g.cpk[:, 4:8]
        g.bv_sb = g.cpk[:, 8 : 8 + JC]
        g.bp_sb = g.cpk[:, 8 + JC : 8 + JC + C]
        g.tri_sb = consts.tile([128, 128], BF16, tag="tri")
        g.ones1 = consts.tile([1, 128], BF16, tag="ones1")

        g.q_sb = [qk_pool.tile([128, T], BF16, tag=f"q{h}", name=f"q{h}") for h in range(HL)]
        g.k_sb = [qk_pool.tile([128, T], BF16, tag=f"k{jt}", name=f"k{jt}") for jt in range(4)]
        g.v_sb = [v_pool.tile([128, HL, 128], BF16, tag=f"v{tt}", name=f"v{tt}") for tt in range(TT)]
        g.y_sb = [y_pool.tile([128, T], BF16, tag=f"y{jt}", name=f"y{jt}") for jt in range(4)]
        # zero-pad the unused half of each per-head Q tile (full-128 contraction
        # keeps the PE array activity high so the HAM clock gate stays warm).
        # All memsets run on gpsimd so the vector queue is free for the early
        # projection drains; order matches first use.
        for h in (0, 1):
            pad = (64, 128) if h % 2 == 0 else (0, 64)
            nc.gpsimd.memset(g.q_sb[h][pad[0] : pad[1], :], 0.0)
        for tt in range(4):
            nc.gpsimd.memset(g.v_sb[tt], 0.0)
            nc.gpsimd.memset(g.v_sb[tt][:, :, 64:65], 1.0)
        for h in range(2, HL):
            pad = (64, 128) if h % 2 == 0 else (0, 64)
            nc.gpsimd.memset(g.q_sb[h][pad[0] : pad[1], :], 0.0)
        for tt in range(4, TT):
            nc.gpsimd.memset(g.v_sb[tt], 0.0)
            nc.gpsimd.memset(g.v_sb[tt][:, :, 64:65], 1.0)
        nc.gpsimd.memset(g.ones1, 1.0)

        with (
            tc.tile_pool(name="wat", bufs=1) as w_pool,
            tc.tile_pool(name="xt", bufs=2) as xt_pool,
        ):
            # ---- initial DMAs: interleave across both HWDGE queues ------
            # scalar queue: wq, wk slices, wp (done before the first exp).
            # sync queue: xt chunks, wv, then bounces + out stores.
            xT_r = xT.rearrange("(ct p) t -> p ct t", p=128)
            g.wq = w_pool.tile([128, CT, JC], BF16, tag="wq")
            g.wk = w_pool.tile([128, CT, JC], BF16, tag="wk")
            g.wv = w_pool.tile([128, CT, JC], BF16, tag="wv")
            g.xt = [
                xt_pool.tile([128, CT, TCH], BF16, tag="xt", name=f"x{c}")
                for c in range(NCH)
            ]
            nc.scalar.dma_start(out=g.wq, in_=wq.rearrange("(ct p) j -> p ct j", p=128))
            nc.scalar.dma_start(out=g.cpk, in_=cpk)
            nc.scalar.dma_start(out=g.tri_sb, in_=tri)
            nc.sync.dma_start(out=g.xt[0], in_=xT_r[:, :, ts(0, TCH)])
            nc.scalar.dma_start(out=g.wk, in_=wk.rearrange("(ct p) j -> p ct j", p=128))
            nc.sync.dma_start(out=g.xt[1], in_=xT_r[:, :, ts(1, TCH)])
            nc.scalar.dma_start(out=g.wv, in_=wv.rearrange("(ct p) j -> p ct j", p=128))
            g.wp_sb = wp_pool.tile([128, 4, C], BF16, tag="wp")
            nc.scalar.dma_start(
                out=g.wp_sb, in_=wp.rearrange("(jt p) c -> p jt c", p=128)
            )
            for c in (2, 3):
                nc.sync.dma_start(out=g.xt[c], in_=xT_r[:, :, ts(c, TCH)])

            # ---- phase A: QK then V for t < 1024 ------------------------
            for c in (0, 1):
                for which in ("q", "k"):
                    for jt in range(4):
                        _proj_group(g, which, c, jt)
            for c in (0, 1):
                for sub in range(TCH // 128):
                    _proj_group(g, "v", c, sub)

            # ---- phase B: attention half 0 + projection fillers ---------
            fillers = []
            for c in (2, 3):
                for which in ("q", "k"):
                    for jt in range(4):
                        fillers.append(
                            (lambda wh=which, cc=c, j=jt: _proj_group(g, wh, cc, j))
                        )
            for c in (2, 3):
                for sub in range(TCH // 128):
                    fillers.append((lambda cc=c, s=sub: _proj_group(g, "v", cc, s)))
            per = (len(fillers) + HL - 1) // HL
            pending = []
            for h in range(HL):
                fl = pending + fillers[h * per : (h + 1) * per]
                pending = [_attn_half(g, h, 0, fl)]

        # ---- phase C: attention half 1 + out-proj t<1024 fillers --------
        for h in range(HL):
            fl = pending + [lambda t0=h: _oproj_tile(g, t0)]
            pending = [_attn_half(g, h, 1, fl)]
        for f in pending:
            f()

        # ---- phase D: out-proj t >= 1024 --------------------------------
        for tt in range(8, 16):
            _oproj_tile(g, tt)


_CACHE = {}


def build_nc():
    if "nc" in _CACHE:
        return _CACHE["nc"]
    nc = bacc.Bacc(
        "TRN2",
        target_bir_lowering=False,
        debug=False,
        enable_asserts=False,
        num_devices=NCORES,
    )
    io = (
        nc.dram_tensor("xT", [C, T], BF16, kind="ExternalInput").ap(),
        nc.dram_tensor("wq", [C, JC], BF16, kind="ExternalInput").ap(),
        nc.dram_tensor("wk", [C, JC], BF16, kind="ExternalInput").ap(),
        nc.dram_tensor("wv", [C, JC], BF16, kind="ExternalInput").ap(),
        nc.dram_tensor("wp", [JC, C], BF16, kind="ExternalInput").ap(),
        nc.dram_tensor("cpk", [128, 4 + 4 + JC + C], F32, kind="ExternalInput").ap(),
        nc.dram_tensor("tri", [128, 128], BF16, kind="ExternalInput").ap(),
        nc.dram_tensor("out", [T, C], F32, kind="ExternalOutput").ap(),
    )
    with tile.TileContext(nc) as tc:
        _trace(nc, tc, io)
    nc.compile()
    _CACHE["nc"] = nc
    return nc


def make_in_maps(x, w_attn, b_attn, w_proj, b_proj):
    import ml_dtypes
    tri = np.triu(np.ones((128, 128), dtype=ml_dtypes.bfloat16))
    zeros_c = np.zeros(C, dtype=np.float32)
    in_maps = []
    for core in range(NCORES):
        b, hh = core // 2, core % 2
        j0 = JC * hh
        bq = b_attn[j0 : j0 + JC].astype(np.float32)
        bk = b_attn[C + j0 : C + j0 + JC].astype(np.float32)
        bv = b_attn[2 * C + j0 : 2 * C + j0 + JC].astype(np.float32)
        bp = b_proj.astype(np.float32) if hh == 0 else zeros_c
        cpk = np.empty((128, 4 + 4 + JC + C), dtype=np.float32)
        cpk[:, 0:4] = bq.reshape(4, 128).T
        cpk[:, 4:8] = bk.reshape(4, 128).T
        cpk[:, 8 : 8 + JC] = bv[None, :]
        cpk[:, 8 + JC :] = bp[None, :]
        in_maps.append(
            {
                "xT": np.ascontiguousarray(x[b].T).astype(ml_dtypes.bfloat16),
                "wq": np.ascontiguousarray(w_attn[:, j0 : j0 + JC]).astype(ml_dtypes.bfloat16),
                "wk": np.ascontiguousarray(w_attn[:, C + j0 : C + j0 + JC]).astype(ml_dtypes.bfloat16),
                "wv": np.ascontiguousarray(w_attn[:, 2 * C + j0 : 2 * C + j0 + JC]).astype(ml_dtypes.bfloat16),
                "wp": np.ascontiguousarray(w_proj[j0 : j0 + JC, :]).astype(ml_dtypes.bfloat16),
                "cpk": cpk,
                "tri": tri,
            }
        )
    return in_maps


def gather(parts):
    out = np.empty((B, T, C), dtype=np.float32)
    for b in range(B):
        out[b] = parts[2 * b]["out"] + parts[2 * b + 1]["out"]
    return out


def kernel(x, w_attn, b_attn, w_proj, b_proj):
    x = np.asarray(x, dtype=np.float32)
    w_attn = np.asarray(w_attn, dtype=np.float32)
    b_attn = np.asarray(b_attn, dtype=np.float32)
    w_proj = np.asarray(w_proj, dtype=np.float32)
    b_proj = np.asarray(b_proj, dtype=np.float32)
    nc = build_nc()
    in_maps = make_in_maps(x, w_attn, b_attn, w_proj, b_proj)
    res = run_bass_kernel_spmd(nc, in_maps, core_ids=list(range(NCORES)))
    return gather(res.results)


if __name__ == "__main__":
    rng = np.random.default_rng(0)
    x = rng.standard_normal((B, T, C), dtype=np.float32)
    w_attn = rng.standard_normal((C, 3 * C), dtype=np.float32) / np.sqrt(C)
    b_attn = np.zeros(3 * C, np.float32)
    w_proj = rng.standard_normal((C, C), dtype=np.float32) / np.sqrt(C)
    b_proj = np.zeros(C, np.float32)
    out = kernel(x, w_attn, b_attn, w_proj, b_proj)
    print(out.shape, out.dtype, np.abs(out).mean())
